# revision 19
# baseline (speedup 1.0000x reference)
"""Trainium2 Bass kernel for a 3x3 stride-1 pad-1 conv, NCHW (16,16,512,512) fp32.

Matches the reference semantics exactly:
  - effective weights: K flattened as (ki,kj,ci) but consumed as (ci,ki,kj):
      Weff[ki,kj,ci,co] = K.reshape(144,16)[ci*9 + ki*3 + kj, co]
  - last output row and column are zero.

Strategy: pure data parallel over the batch (2 images per core on 8 cores),
weights replicated.

DMA design (the bottleneck is the 16 shared SDMA engines, ~27 GiB/s each):
  - the host repacks x into fp16 in the exact SBUF tile layout the matmuls
    consume: partition (hi,ci) = hi*16+ci holds, for each of 86 row-groups g
    and both images, the 520-col padded row starts[g]-1+hi (col 0 = left pad,
    1..512 = data, 513 = right pad, rest zero slack). Row -1 and the group
    overlap duplication are baked in host-side, so a chunk of 11 groups loads
    as ONE dma with 128 contiguous ~23KB per-partition descriptors;
  - output is stored as fp16 in a raw [96, 86*1024] group-major layout
    (~22KB per-partition descriptors per chunk store); the host unscrambles
    to NCHW, zeroes the masked last row/col, and casts to fp32;
  - all big DMAs are issued on the scalar (Act) HWDGE ring, which fans out to
    all 16 SDMA engines (the sync ring only reaches 8, SWDGE only 6);
  - chunks alternate psum/out partition offset 0/32 so the 96-partition
    stores load even/odd SDMA engines evenly.

Compute: banded fp16 matmuls, contraction K = 8 input rows x 16 c_in = 128
partitions, M = 6 out rows x 16 c_out = 96, one matmul per (group, image,
kj) with N = 512; accumulation in fp32 PSUM ([128,1024] = 2 banks per
group); PSUM->SBUF fp16 copies alternate between the vector and scalar
engines so neither binds.
"""

import numpy as np

import concourse.bass as bass
import concourse.mybir as mybir
import concourse.tile as tile
from concourse import bacc
from concourse.bass_utils import run_bass_kernel_spmd

F32 = mybir.dt.float32
F16 = mybir.dt.float16

C = 16  # channels (in == out)
W = 512  # image width
H = 512  # image height
R = 6  # output rows per matmul group
M = R * C  # matmul output partitions (96)
GW = 520  # padded group-row width: col 0 pad, 1..512 data, 513 pad, 514.. slack
NIMG = 2  # images per core
N_CORES = 8
NG = 86  # row groups per image: starts 0,6,...,504 plus overlapped 505
STARTS = [6 * g for g in range(85)] + [505]
_CHUNK_SIZES = [1, 2, 4, 7, 10, 11, 11, 11, 11, 6, 4, 3, 2, 2, 1]
CHUNKS = []
_g = 0
for _s in _CHUNK_SIZES:
    CHUNKS.append((_g, _s))
    _g += _s
assert _g == NG


def _weff(K: np.ndarray) -> np.ndarray:
    Kflat = K.reshape(9 * C, C).astype(np.float32)
    Weff = np.zeros((3, 3, C, C), np.float32)
    for ki in range(3):
        for kj in range(3):
            for ci in range(C):
                Weff[ki, kj, ci, :] = Kflat[ci * 9 + ki * 3 + kj, :]
    return Weff


def _build_banded_weights(K: np.ndarray):
    """lhsT matrices [3, 128, 96] fp16, partition k = hi*16+ci, m = ho*16+co,
    ki = hi - ho."""
    Weff = _weff(K)
    wa_hi = np.zeros((3, 128, M), np.float32)
    for kj in range(3):
        for ho in range(R):
            for ki in range(3):
                hi = ho + ki
                blk = Weff[ki, kj]  # [ci, co]
                for ci in range(C):
                    wa_hi[kj, hi * C + ci, ho * C:(ho + 1) * C] = blk[ci]
    return wa_hi.astype(np.float16)


def build_nc(in_bufs: int = 5, out_bufs: int = 3, psum_bufs: int = 8,
             lookahead: int = 2):
    nc = bacc.Bacc(None, target_bir_lowering=False)
    xs = nc.dram_tensor("xs", [128, NG * NIMG * GW], F16, kind="ExternalInput")
    whi = nc.dram_tensor("whi", [3, 128, M], F16, kind="ExternalInput")
    ys = nc.dram_tensor("ys", [M, NG * 1024], F16, kind="ExternalOutput")

    with tile.TileContext(nc) as tc:
        with (
            tc.tile_pool(name="wpool", bufs=1) as wpool,
            tc.tile_pool(name="inpool", bufs=in_bufs) as inpool,
            tc.tile_pool(name="outpool", bufs=out_bufs) as outpool,
            tc.tile_pool(name="psum", bufs=psum_bufs, space="PSUM") as psum_pool,
        ):
            # weights go on the same Act HWDGE ring as the bulk traffic and
            # are issued FIRST: the Act ring strictly outprioritizes the sync
            # ring on the shared SDMA engines, so a sync-ring weights load
            # would drain dead last and stall the first LDWEIGHTS for ~25us.
            whi_t = wpool.tile([128, 3, M], F16)
            nc.scalar.dma_start(
                whi_t[:], bass.AP(whi, 0, [[M, 128], [128 * M, 3], [1, M]])
            )

            def load_chunk(ci):
                g0, gc = CHUNKS[ci]
                t = inpool.tile([128, gc * NIMG * GW], F16, name=f"in_{ci}",
                                tag="in")
                src = bass.AP(xs, g0 * NIMG * GW,
                              [[NG * NIMG * GW, 128], [1, gc * NIMG * GW]])
                # head loads go via SWDGE: it drains concurrently with the
                # Act ring and its 6 engines exclude the straggling E79, so
                # the first matmul isn't held ~2us past the bulk drain
                eng = nc.gpsimd if ci < 2 else nc.scalar
                eng.dma_start(t[:], src)
                return t

            def compute_chunk(ci, t):
                g0, gc = CHUNKS[ci]
                off = 0
                out_t = outpool.tile([128, gc * 1024], F16, name=f"out_{ci}",
                                     tag="out")
                for j in range(gc):
                    for img in range(NIMG):
                        ps = psum_pool.tile([128, 512], F32,
                                            name=f"ps_{ci}_{j}_{img}",
                                            tag="ps")
                        base = (j * NIMG + img) * GW
                        for kj in range(3):
                            nc.tensor.matmul(
                                ps[off:off + M, :],
                                whi_t[:, kj, :],
                                t[:, base + kj:base + kj + 512],
                                start=(kj == 0), stop=(kj == 2),
                            )
                        # each psum drains via both engines at once so the
                        # copy service rate stays ahead of the matmul pace
                        c0 = (j * NIMG + img) * 512
                        nc.vector.tensor_copy(out_t[off:off + M, c0:c0 + 256],
                                              ps[off:off + M, 0:256])
                        nc.scalar.copy(out_t[off:off + M, c0 + 256:c0 + 512],
                                       ps[off:off + M, 256:512])
                store_dst = bass.AP(ys, g0 * 1024,
                                    [[NG * 1024, M], [1, gc * 1024]])
                # final store via SWDGE: drains in parallel with the Act
                # ring's penultimate store and skips the E79 straggler
                eng = nc.gpsimd if ci == len(CHUNKS) - 1 else nc.scalar
                eng.dma_start(store_dst, out_t[off:off + M, :])

            pend = []
            for idx in range(len(CHUNKS) + lookahead):
                if idx < len(CHUNKS):
                    pend.append(load_chunk(idx))
                if idx >= lookahead:
                    compute_chunk(idx - lookahead, pend[idx - lookahead])

    nc.finalize()
    return nc


def _pack_inputs(x16: np.ndarray) -> np.ndarray:
    """x16: [NIMG, C, H, W] fp16 for one core -> [128, NG*NIMG*GW]."""
    xarr = np.zeros((128, NG, NIMG, GW), np.float16)
    starts = np.asarray(STARTS)
    for hi in range(8):
        rows = np.clip(starts - 1 + hi, 0, H - 1)
        blk = x16[:, :, rows, :]  # [NIMG, C, NG, W]
        xarr[hi * 16:(hi + 1) * 16, :, :, 1:1 + W] = blk.transpose(1, 2, 0, 3)
    xarr[0:16, 0, :, :] = 0  # row -1 above the top edge
    return np.ascontiguousarray(xarr.reshape(128, NG * NIMG * GW))


def _unpack_output(ys_raw: np.ndarray) -> np.ndarray:
    """ys_raw: [M, NG*1024] fp16 for one core -> [NIMG, C, H, W] fp16."""
    arr = ys_raw.reshape(R, C, NG, NIMG, W)  # (ho, co, g, img, col)
    arr = arr.transpose(3, 1, 2, 0, 4)  # (img, co, g, ho, col)
    y = np.zeros((NIMG, C, H, W), np.float16)
    for g, s in enumerate(STARTS):
        y[:, :, s:s + R, :] = arr[:, :, g]
    y[:, :, :, W - 1] = 0  # masked last column; last row never written
    return y


def _run(x: np.ndarray, K: np.ndarray, core_ids, trace=False, **kw):
    """x: [n_total, C, H, W] fp32, split evenly over core_ids."""
    n_cores = len(core_ids)
    n_total = x.shape[0]
    assert n_total == n_cores * NIMG
    wa_hi = _build_banded_weights(K)
    x16 = x.astype(np.float16)
    nc = build_nc(**kw)
    in_maps = [
        {"xs": _pack_inputs(x16[i * NIMG:(i + 1) * NIMG]), "whi": wa_hi}
        for i in range(n_cores)
    ]
    res = run_bass_kernel_spmd(nc, in_maps, core_ids=list(core_ids),
                               trace=trace)
    y16 = np.concatenate([_unpack_output(r["ys"]) for r in res.results],
                         axis=0)
    return y16.astype(np.float32), res


def kernel(**inputs) -> np.ndarray:
    x = np.ascontiguousarray(np.asarray(inputs["x"], dtype=np.float32))
    K = np.ascontiguousarray(np.asarray(inputs["K"], dtype=np.float32))
    y, _ = _run(x, K, core_ids=range(N_CORES))
    return y


# revision 21
# speedup vs baseline: 1.0485x; 1.0485x over previous
"""Trainium2 Bass kernel for a 3x3 stride-1 pad-1 conv, NCHW (16,16,512,512) fp32.

Matches the reference semantics exactly:
  - effective weights: K flattened as (ki,kj,ci) but consumed as (ci,ki,kj):
      Weff[ki,kj,ci,co] = K.reshape(144,16)[ci*9 + ki*3 + kj, co]
  - last output row and column are zero.

Strategy: pure data parallel over the batch (2 images per core on 8 cores),
weights replicated.

DMA design (the bottleneck is the 16 shared SDMA engines, ~27 GiB/s each):
  - the host repacks x into fp16 in the exact SBUF tile layout the matmuls
    consume: partition (hi,ci) = hi*16+ci holds, for each of 86 row-groups g
    and both images, the 520-col padded row starts[g]-1+hi (col 0 = left pad,
    1..512 = data, 513 = right pad, rest zero slack). Row -1 and the group
    overlap duplication are baked in host-side, so a chunk of 11 groups loads
    as ONE dma with 128 contiguous ~23KB per-partition descriptors;
  - output is stored as fp16 in a raw [96, 86*1024] group-major layout
    (~22KB per-partition descriptors per chunk store); the host unscrambles
    to NCHW, zeroes the masked last row/col, and casts to fp32;
  - all big DMAs are issued on the scalar (Act) HWDGE ring, which fans out to
    all 16 SDMA engines (the sync ring only reaches 8, SWDGE only 6);
  - chunks alternate psum/out partition offset 0/32 so the 96-partition
    stores load even/odd SDMA engines evenly.

Compute: banded fp16 matmuls, contraction K = 8 input rows x 16 c_in = 128
partitions, M = 6 out rows x 16 c_out = 96, one matmul per (group, image,
kj) with N = 512; accumulation in fp32 PSUM ([128,1024] = 2 banks per
group); PSUM->SBUF fp16 copies alternate between the vector and scalar
engines so neither binds.
"""

import numpy as np

import concourse.bass as bass
import concourse.mybir as mybir
import concourse.tile as tile
from concourse import bacc
from concourse.bass_utils import run_bass_kernel_spmd

F32 = mybir.dt.float32
F16 = mybir.dt.float16

C = 16  # channels (in == out)
W = 512  # image width
H = 512  # image height
R = 6  # output rows per matmul group
M = R * C  # matmul output partitions (96)
GW = 520  # padded group-row width: col 0 pad, 1..512 data, 513 pad, 514.. slack
NIMG = 2  # images per core
N_CORES = 8
NG = 86  # row groups per image: starts 0,6,...,504 plus overlapped 505
STARTS = [6 * g for g in range(85)] + [505]
_CHUNK_SIZES = [1, 2, 4, 7, 10, 11, 11, 11, 11, 6, 4, 3, 2, 2, 1]
CHUNKS = []
_g = 0
for _s in _CHUNK_SIZES:
    CHUNKS.append((_g, _s))
    _g += _s
assert _g == NG


def _weff(K: np.ndarray) -> np.ndarray:
    Kflat = K.reshape(9 * C, C).astype(np.float32)
    Weff = np.zeros((3, 3, C, C), np.float32)
    for ki in range(3):
        for kj in range(3):
            for ci in range(C):
                Weff[ki, kj, ci, :] = Kflat[ci * 9 + ki * 3 + kj, :]
    return Weff


def _build_banded_weights(K: np.ndarray):
    """lhsT matrices [3, 128, 96] fp16, partition k = hi*16+ci, m = ho*16+co,
    ki = hi - ho."""
    Weff = _weff(K)
    wa_hi = np.zeros((3, 128, M), np.float32)
    for kj in range(3):
        for ho in range(R):
            for ki in range(3):
                hi = ho + ki
                blk = Weff[ki, kj]  # [ci, co]
                for ci in range(C):
                    wa_hi[kj, hi * C + ci, ho * C:(ho + 1) * C] = blk[ci]
    return wa_hi.astype(np.float16)


def build_nc(in_bufs: int = 5, out_bufs: int = 3, psum_bufs: int = 8,
             lookahead: int = 2):
    nc = bacc.Bacc(None, target_bir_lowering=False)
    xs = nc.dram_tensor("xs", [128, NG * NIMG * GW], F16, kind="ExternalInput")
    whi = nc.dram_tensor("whi", [3, 128, M], F16, kind="ExternalInput")
    ys = nc.dram_tensor("ys", [M, NG * 1024], F16, kind="ExternalOutput")

    with tile.TileContext(nc) as tc:
        with (
            tc.tile_pool(name="wpool", bufs=1) as wpool,
            tc.tile_pool(name="inpool", bufs=in_bufs) as inpool,
            tc.tile_pool(name="outpool", bufs=out_bufs) as outpool,
            tc.tile_pool(name="psum", bufs=psum_bufs, space="PSUM") as psum_pool,
        ):
            # weights go on the same Act HWDGE ring as the bulk traffic and
            # are issued FIRST: the Act ring strictly outprioritizes the sync
            # ring on the shared SDMA engines, so a sync-ring weights load
            # would drain dead last and stall the first LDWEIGHTS for ~25us.
            whi_t = wpool.tile([128, 3, M], F16)
            nc.scalar.dma_start(
                whi_t[:], bass.AP(whi, 0, [[M, 128], [128 * M, 3], [1, M]])
            )

            def load_chunk(ci):
                g0, gc = CHUNKS[ci]
                t = inpool.tile([128, gc * NIMG * GW], F16, name=f"in_{ci}",
                                tag="in")
                src = bass.AP(xs, g0 * NIMG * GW,
                              [[NG * NIMG * GW, 128], [1, gc * NIMG * GW]])
                nc.scalar.dma_start(t[:], src)
                return t

            def compute_chunk(ci, t):
                g0, gc = CHUNKS[ci]
                off = 0
                out_t = outpool.tile([128, gc * 1024], F16, name=f"out_{ci}",
                                     tag="out")
                for j in range(gc):
                    for img in range(NIMG):
                        ps = psum_pool.tile([128, 512], F32,
                                            name=f"ps_{ci}_{j}_{img}",
                                            tag="ps")
                        base = (j * NIMG + img) * GW
                        for kj in range(3):
                            nc.tensor.matmul(
                                ps[off:off + M, :],
                                whi_t[:, kj, :],
                                t[:, base + kj:base + kj + 512],
                                start=(kj == 0), stop=(kj == 2),
                            )
                        # each psum drains via both engines at once so the
                        # copy service rate stays ahead of the matmul pace
                        c0 = (j * NIMG + img) * 512
                        nc.vector.tensor_copy(out_t[off:off + M, c0:c0 + 256],
                                              ps[off:off + M, 0:256])
                        nc.scalar.copy(out_t[off:off + M, c0 + 256:c0 + 512],
                                       ps[off:off + M, 256:512])
                store_dst = bass.AP(ys, g0 * 1024,
                                    [[NG * 1024, M], [1, gc * 1024]])
                nc.scalar.dma_start(store_dst, out_t[off:off + M, :])

            pend = []
            for idx in range(len(CHUNKS) + lookahead):
                if idx < len(CHUNKS):
                    pend.append(load_chunk(idx))
                if idx >= lookahead:
                    compute_chunk(idx - lookahead, pend[idx - lookahead])

    nc.finalize()
    return nc


def _pack_inputs(x16: np.ndarray) -> np.ndarray:
    """x16: [NIMG, C, H, W] fp16 for one core -> [128, NG*NIMG*GW]."""
    xarr = np.zeros((128, NG, NIMG, GW), np.float16)
    starts = np.asarray(STARTS)
    for hi in range(8):
        rows = np.clip(starts - 1 + hi, 0, H - 1)
        blk = x16[:, :, rows, :]  # [NIMG, C, NG, W]
        xarr[hi * 16:(hi + 1) * 16, :, :, 1:1 + W] = blk.transpose(1, 2, 0, 3)
    xarr[0:16, 0, :, :] = 0  # row -1 above the top edge
    return np.ascontiguousarray(xarr.reshape(128, NG * NIMG * GW))


def _unpack_output(ys_raw: np.ndarray) -> np.ndarray:
    """ys_raw: [M, NG*1024] fp16 for one core -> [NIMG, C, H, W] fp16."""
    arr = ys_raw.reshape(R, C, NG, NIMG, W)  # (ho, co, g, img, col)
    arr = arr.transpose(3, 1, 2, 0, 4)  # (img, co, g, ho, col)
    y = np.zeros((NIMG, C, H, W), np.float16)
    for g, s in enumerate(STARTS):
        y[:, :, s:s + R, :] = arr[:, :, g]
    y[:, :, :, W - 1] = 0  # masked last column; last row never written
    return y


def _run(x: np.ndarray, K: np.ndarray, core_ids, trace=False, **kw):
    """x: [n_total, C, H, W] fp32, split evenly over core_ids."""
    n_cores = len(core_ids)
    n_total = x.shape[0]
    assert n_total == n_cores * NIMG
    wa_hi = _build_banded_weights(K)
    x16 = x.astype(np.float16)
    nc = build_nc(**kw)
    in_maps = [
        {"xs": _pack_inputs(x16[i * NIMG:(i + 1) * NIMG]), "whi": wa_hi}
        for i in range(n_cores)
    ]
    res = run_bass_kernel_spmd(nc, in_maps, core_ids=list(core_ids),
                               trace=trace)
    y16 = np.concatenate([_unpack_output(r["ys"]) for r in res.results],
                         axis=0)
    return y16.astype(np.float32), res


def kernel(**inputs) -> np.ndarray:
    x = np.ascontiguousarray(np.asarray(inputs["x"], dtype=np.float32))
    K = np.ascontiguousarray(np.asarray(inputs["K"], dtype=np.float32))
    y, _ = _run(x, K, core_ids=range(N_CORES))
    return y
